# revision 1
# baseline (speedup 1.0000x reference)
"""Trainium2 Bass kernel for nn_DAMASK_GCN (attention-weighted 3-way GEMM + sigmoid).

Math: rating[n,m] = sigmoid( sum_k r_k*w_k ) with w = softmax_k(a_k),
a_k[n,m] = u_k[n].h_u + i_k[m].h_i  (separable!), r_k = u_k @ i_k^T.

Separable-softmax trick: exp(a_k) = exp(alpha_k[n]) * exp(beta_k[m]), so with
scaled embeddings  u~_k = exp(alpha_k)*u_k,  i~_k = exp(beta_k)*i_k :
    S1 = sum_k u~_k @ i~_k^T          (one K=192 GEMM)
    S0 = sum_k exp(alpha_k) exp(beta_k)^T   (one K=3 GEMM)
    rating = sigmoid(S1 / S0)
No per-cell softmax needed; everything is GEMM + cheap elementwise.

Sharding: users split across 8 cores (512 rows each); items + h replicated.
"""

import os

import numpy as np

N_USERS, N_ITEMS, DIM = 4096, 8192, 64
N_CORES = 8
UL = N_USERS // N_CORES  # 512 users per core
P = 128
NT = UL // P             # 4 user tiles per core
TS_I = N_ITEMS // P      # 64 item subtiles
GRP = 16                 # item subtiles per prep group
NG = TS_I // GRP         # 4 prep groups
QUAD = 2048              # m-columns per main-loop quad
NQ = N_ITEMS // QUAD     # 4 quads
CH = 512                 # matmul moving free dim

_CACHE = {}
USE_FUSED_DIV = False  # single custom-DVE op out = in1 * approx(1/in0) (~1.7e-3 rel)


def _register_div_op():
    """Register a fused divide custom-DVE op: out = Src1 * recip_1NR(Src0).

    Same bitwise-NOT seed + one Newton pass as RECIPROCAL_APPROX_FAST (which
    uses two passes in 8/8 stages — no room for the extra multiply), then the
    multiply by Src1. 6 stages, one DVE instruction instead of recip+mul.
    """
    if "div_op" in _CACHE:
        return _CACHE["div_op"]
    import numpy as np

    from concourse import dve_ops as DO
    from concourse.dve_spec import AluOp, Bin, C0, C1, Spec, Src0, Src1, lower
    from concourse.dve_uop import DveOpSpec

    name = "DIV_APPROX_ANT"
    _not_x = Bin(AluOp.BITWISE_NOT, Src0, Src0)
    _y0 = _not_x * C0
    _y1 = _y0 * (C1 - Src0 * _y0)

    def _ref(in0, in1, c0, c1, c2):
        not_x = (~in0.view(np.int32)).view(np.float32)
        y0 = not_x * c0
        y1 = y0 * (c1 - in0 * y0)
        return in1 * y1

    spec = Spec(body=Src1 * _y1, reference=_ref)
    row = max(DO._SUB_OPCODE_FOR_NAME.values()) + 1
    assert row < 0x20
    shas = {}
    for ver in ("v3", "v4"):
        uops = lower(spec, ver=ver)
        shas[ver] = DveOpSpec(name=name, opcode=row, uops=uops, rd1_en=True).sha(ver)
    op = DO.DveOp(name, spec, subdim=False, uops_sha=shas)
    if not any(o.name == name for o in DO.OPS):
        DO.OPS.append(op)
        DO._SUB_OPCODE_FOR_NAME[name] = row
    _CACHE["div_op"] = op
    return op


def _emit(tc, nc, out_ap, u_aps, i_aps, h_ap):
    """Per-core program.

    Walrus's LDWEIGHTS struct encodes only ONE sync wait, and Tile does not
    exploit cross-engine transitivity -- so the structure below is arranged so
    every matmul needs at most one new semaphore wait:
      * all input DMAs are issued up front;
      * scaled embeddings live in fresh tiles whose only writers are DVE ops
        (scale multiply, then an exp-column copy as the single DVE funnel);
      * each prep group's PSUM slot-release (an ACT copy) is absorbed by a
        dummy identity transpose before the real transposes.
    """
    from contextlib import ExitStack

    import concourse.mybir as mybir
    from concourse.masks import make_identity

    f32 = mybir.dt.float32
    AF = mybir.ActivationFunctionType
    AX = mybir.AxisListType

    with ExitStack() as ctx:
        pers = ctx.enter_context(tc.tile_pool(name="pers", bufs=1))
        workA = ctx.enter_context(tc.tile_pool(name="workA", bufs=1))
        work = ctx.enter_context(tc.tile_pool(name="work", bufs=2))
        psum = ctx.enter_context(tc.tile_pool(name="psum", bufs=1, space="PSUM"))

        # ---- persistent tensors ----
        f32r = mybir.dt.float32r
        A1 = pers.tile([P, N_ITEMS], f32r)       # [i1~^T ; i2~^T] rows 0-127
        A2 = pers.tile([67, N_ITEMS], f32r)      # [i3~^T (64) ; ei1 ; ei2 ; ei3]
        B1 = pers.tile([P, UL], f32r)            # [u1~^T ; u2~^T]
        B2 = pers.tile([67, UL], f32r)           # [u3~^T (64) ; eu1 ; eu2 ; eu3]
        nat_u = pers.tile([P, NT, 192], f32)
        scl_u = pers.tile([P, NT, 195], f32r)    # scaled users + eu cols
        ident = pers.tile([P, P], f32)
        identr = pers.tile([P, P], f32r)
        hrep = pers.tile([P, 2 * DIM], f32)      # h broadcast to all partitions
        hsb = pers.tile([1, 2 * DIM], f32)
        ones1 = pers.tile([1, P], f32)

        # ---- phase 0: all input DMAs + constants ----
        from concourse.tile_rust import add_dep_helper

        nc.sync.dma_start(hsb[:], h_ap.rearrange("(a b) -> a b", a=1))
        _udmas = []
        for k in range(3):
            d = nc.sync.dma_start(
                nat_u[:, :, k * DIM : (k + 1) * DIM],
                u_aps[k].rearrange("(t p) d -> p t d", p=P),
            )
            _udmas.append(d.ins)
        nc.gpsimd.memset(ones1[:], 1.0)
        make_identity(nc, ident[:])

        # prime PE's Pool tick (identity/ones writers), then h-broadcast.
        f32r = mybir.dt.float32r
        ps_h = psum.tile([P, P], f32, tag="psB")
        nc.tensor.transpose(ps_h[:], ident[:], ident[:])  # dummy
        nc.tensor.matmul(ps_h[:], lhsT=ones1[:], rhs=hsb[:], start=True, stop=True)
        nc.scalar.copy(hrep[:], ps_h[:])
        nc.scalar.copy(identr[:], ident[:])

        hu_b = hrep[:, None, None, 0:DIM]
        hi_b = hrep[:, None, None, DIM : 2 * DIM]

        def prep_group(nat, scl, n_sub, gs, h_b, psA_w, psB_w, first_psum_dummy,
                       join_deps=()):
            """mult -> reduce -> exp -> scale -> ei-copy -> transposes.
            Returns (psA, psB) holding the transposed group."""
            for di in join_deps:
                jd = nc.vector.drain(fusable=False)
                add_dep_helper(jd.ins, di, sync=True, reason="dma joiner")
            nat4 = nat[:, gs, :].rearrange("p t (k d) -> p t k d", k=3)
            tmp = workA.tile([P, n_sub, 3, DIM], f32, tag="tmp")
            nc.vector.tensor_mul(tmp[:], nat4, h_b.to_broadcast([P, n_sub, 3, DIM]))
            bt = workA.tile([P, n_sub, 3], f32, tag="bt")
            nc.vector.reduce_sum(bt[:], tmp[:], axis=AX.X)
            bt2 = workA.tile([P, n_sub, 3], f32, tag="bt2")
            nc.scalar.activation(bt2[:], bt[:], AF.Exp)
            scl4 = scl[:, gs, 0:192].rearrange("p t (k d) -> p t k d", k=3)
            nc.vector.tensor_mul(
                scl4, nat4, bt2[:][:, :, :, None].to_broadcast([P, n_sub, 3, DIM])
            )
            # single DVE funnel: after this, scl is fully written by DVE only
            nc.vector.tensor_copy(scl[:, gs, 192:195], bt2[:])

            psA = psum.tile([P, psA_w], f32r, tag="psA")
            psB = psum.tile([P, psB_w], f32r, tag="psB")
            if first_psum_dummy:
                # absorb the ACT slot-release wait on PE before real work
                nc.tensor.transpose(psB[:, 0:P], identr[:], identr[:])
            for j in range(n_sub):
                t = gs.start + j
                nc.tensor.transpose(
                    psB[0:67, j * P : (j + 1) * P], scl[:, t, 128:195],
                    identr[:],
                )
                nc.tensor.transpose(
                    psA[:, j * P : (j + 1) * P], scl[:, t, 0:128],
                    identr[:],
                )
            return psA, psB

        # ---- user prep ----
        psU, psU2 = prep_group(
            nat_u, scl_u, NT, slice(0, NT), hu_b, CH, CH, first_psum_dummy=True,
            join_deps=_udmas,
        )
        nc.scalar.copy(B1[:], psU[:])
        nc.scalar.copy(B2[:], psU2[0:67, :])

        # ---- item prep (NG groups) ----
        _last_tp = [None] * NG
        for g in range(NG):
            scl_i = work.tile([P, GRP, 195], f32r, tag="scl_i")
            nat_g = work.tile([P, GRP, 192], f32, tag="nat_g")
            gs_nat = slice(g * GRP, (g + 1) * GRP)
            _idmas = []
            for k in range(3):
                d = nc.sync.dma_start(
                    nat_g[:, :, k * DIM : (k + 1) * DIM],
                    i_aps[k].rearrange("(t p) d -> p t d", p=P)[:, gs_nat, :],
                )
                _idmas.append(d.ins)
            if g >= 2 and _last_tp[g - 2] is not None:
                _idmas.append(_last_tp[g - 2])
            for di in _idmas:
                jd = nc.vector.drain(fusable=False)
                add_dep_helper(jd.ins, di, sync=True, reason="group joiner")

            nat4 = nat_g[:, :, :].rearrange("p t (k d) -> p t k d", k=3)
            tmp = workA.tile([P, GRP, 3, DIM], f32, tag="tmp")
            nc.vector.tensor_mul(tmp[:], nat4, hi_b.to_broadcast([P, GRP, 3, DIM]))
            bt = workA.tile([P, GRP, 3], f32, tag="bt")
            nc.vector.reduce_sum(bt[:], tmp[:], axis=AX.X)
            bt2 = workA.tile([P, GRP, 3], f32, tag="bt2")
            nc.scalar.activation(bt2[:], bt[:], AF.Exp)
            scl4 = scl_i[:, :, 0:192].rearrange("p t (k d) -> p t k d", k=3)
            nc.vector.tensor_mul(
                scl4, nat4, bt2[:][:, :, :, None].to_broadcast([P, GRP, 3, DIM])
            )
            nc.vector.tensor_copy(scl_i[:, :, 192:195], bt2[:])

            psA = psum.tile([P, QUAD], f32r, tag="psA")
            psB = psum.tile([P, QUAD], f32r, tag="psB")
            nc.tensor.transpose(psB[:, 0:P], identr[:], identr[:])  # slot absorber
            for j in range(GRP):
                nc.tensor.transpose(
                    psB[0:67, j * P : (j + 1) * P], scl_i[:, j, 128:195],
                    identr[:],
                )
                tp = nc.tensor.transpose(
                    psA[:, j * P : (j + 1) * P], scl_i[:, j, 0:128],
                    identr[:],
                )
            _last_tp[g] = tp.ins
            nc.scalar.copy(A1[:, g * QUAD : (g + 1) * QUAD], psA[:])
            nc.scalar.copy(A2[:, g * QUAD : (g + 1) * QUAD], psB[0:67, :])

        # ---- main loop ----
        _sig_ins = []
        _dma_ins = []
        for t in range(NT):
            tsl = slice(t * P, (t + 1) * P)
            b1 = B1[:, tsl]
            b2e = B2[0:64, tsl]
            b2s = B2[64:67, tsl]
            for q in range(NQ):
                ps0 = psum.tile([P, QUAD], f32, tag="psB")
                for c in range(QUAD // CH):
                    off = q * QUAD + c * CH
                    nc.tensor.matmul(
                        ps0[:, c * CH : (c + 1) * CH],
                        lhsT=b2s,
                        rhs=A2[64:67, off : off + CH],
                        start=True,
                        stop=True,
                    )
                r0 = workA.tile([P, QUAD], f32, tag="r0")
                nc.vector.reciprocal(r0[:], ps0[:])
                ps1 = psum.tile([P, QUAD], f32, tag="psA")
                for c in range(QUAD // CH):
                    off = q * QUAD + c * CH
                    cs = slice(c * CH, (c + 1) * CH)
                    nc.tensor.matmul(
                        ps1[:, cs], lhsT=b1, rhs=A1[:, off : off + CH],
                        start=True, stop=False,
                    )
                    nc.tensor.matmul(
                        ps1[:, cs], lhsT=b2e, rhs=A2[0:64, off : off + CH],
                        start=False, stop=True,
                    )
                qi = t * NQ + q
                if qi >= 2:
                    jd = nc.vector.drain(fusable=False)
                    add_dep_helper(jd.ins, _sig_ins[qi - 2], sync=True, reason="tq slot")
                    ja = nc.scalar.drain(fusable=False)
                    add_dep_helper(ja.ins, _dma_ins[qi - 2], sync=True, reason="oq slot")
                tq = work.tile([P, QUAD], f32, tag="tq")
                nc.vector.tensor_mul(tq[:], ps1[:], r0[:])
                oq = work.tile([P, QUAD], f32, tag="oq")
                s = nc.scalar.activation(oq[:], tq[:], AF.Sigmoid)
                _sig_ins.append(s.ins)
                d = nc.sync.dma_start(out_ap[tsl, q * QUAD : (q + 1) * QUAD], oq[:])
                _dma_ins.append(d.ins)

        # tail: let SP observe every DMA queue's final tick so the framework
        # drain's multi-queue waits prune to <=1 (walrus wait-slot limit)
        for di in _dma_ins[-8:]:
            jd = nc.sync.drain(fusable=False)
            add_dep_helper(jd.ins, di, sync=True, reason="tail dma observe")


def _prune_redundant_waits(nc, same_engine=False):
    """Transitive wait elimination.

    Walrus can encode only ONE sync wait on a Matmult (all waits land on the
    LDWEIGHTS struct), and Tile's per-instruction wait assignment is not
    transitively minimal across engines. Nearly every extra wait here is
    implied: e.g. a matmul waiting {ACT>=c, PE>=t} where the ACT copy at tick c
    itself waited PE>=t. This pass simulates the scheduled stream with vector
    clocks and deletes waits that are (a) transitively implied by kept waits /
    the engine stream, or (b) same-engine completion waits on in-order
    pipelined engines (PE/DVE/ACT), whose writes are ordered by hardware.
    """
    import bisect

    import concourse.mybir as mybir

    fn = nc.m.functions[0]
    insts = [i for blk in fn.blocks for i in blk.instructions]
    idx_of = {inst.name: n for n, inst in enumerate(insts)}

    # sems ever decremented/reset are untouchable (drain-tail butterfly).
    # Our own appended tail resets (ant_sem_reset_*) are after the final
    # barrier and don't affect reasoning — skip them here.
    insts = [i for i in insts if not i.name.startswith("ant_sem_reset")]
    blacklist = set()
    for inst in insts:
        si = inst.sync_info
        if si is None:
            continue
        for u in si.on_update or []:
            if u.update_mode not in ("sem-inc", "sem-add-imm"):
                blacklist.add(u.ant_name)

    # sem event lists: name -> ([cum_value...], [inst_idx...]) in stream order
    events = {}
    cum = {}
    for n, inst in enumerate(insts):
        si = inst.sync_info
        if si is None:
            continue
        for u in si.on_update or []:
            name = u.ant_name
            if name in blacklist:
                continue
            cum[name] = cum.get(name, 0) + u.update_value
            events.setdefault(name, ([], []))
            events[name][0].append(cum[name])
            events[name][1].append(n)

    def guarantor(sem, v):
        ev = events.get(sem)
        if ev is None:
            return None
        k = bisect.bisect_left(ev[0], v)
        if k >= len(ev[0]):
            return None
        return ev[1][k]

    SAME_ENGINE_OK = {"PE": "EngineType.PE", "DVE": "EngineType.DVE",
                      "Activation": "EngineType.Activation",
                      "SP": "EngineType.SP"}

    def sem_engine(sem):
        return sem.rsplit("_", 1)[0]

    stream_clock = {}      # engine -> dict(sem -> guaranteed value at next start)
    completion = [None] * len(insts)  # per-inst completion clock

    def join(dst, src):
        for k, v in src.items():
            if dst.get(k, 0) < v:
                dst[k] = v

    n_del = 0
    for n, inst in enumerate(insts):
        eng = str(inst.engine)
        start = dict(stream_clock.get(eng, {}))
        si = inst.sync_info
        kept = []
        if si is not None and si.on_wait:
            waits = list(si.on_wait)
            # resolve guarantors; process latest-guarantor-first so broad
            # clocks are merged before testing narrower waits
            gids = [(w, guarantor(w.ant_name, w.wait_value)) for w in waits]
            gids.sort(key=lambda t: -1 if t[1] is None else -t[1])
            for w, g in gids:
                if w.ant_name in blacklist:
                    kept.append(w)  # non-monotonic sem: never reason about it
                    continue
                if start.get(w.ant_name, 0) >= w.wait_value:
                    n_del += 1
                    continue
                se = sem_engine(w.ant_name)
                if (
                    same_engine
                    and SAME_ENGINE_OK.get(se) == eng
                    and g is not None
                    and g < n
                ):
                    # in-order pipelined engine: own-queue completion order
                    # makes this wait redundant on hardware (CoreSim's race
                    # model still wants it, so this runs on the HW copy only)
                    if completion[g] is not None:
                        join(start, completion[g])
                    start[w.ant_name] = max(start.get(w.ant_name, 0), w.wait_value)
                    n_del += 1
                    continue
                kept.append(w)
                if g is not None and g < n and completion[g] is not None:
                    join(start, completion[g])
                start[w.ant_name] = max(start.get(w.ant_name, 0), w.wait_value)
            if len(kept) != len(waits):
                inst.sync_info = mybir.SyncInfo(on_wait=kept, on_update=si.on_update)
        comp = dict(start)
        if si is not None:
            for u in si.on_update or []:
                comp[u.ant_name] = max(comp.get(u.ant_name, 0), cum_at(events, u.ant_name, n))
        completion[n] = comp
        stream_clock[eng] = start
    return n_del


def cum_at(events, sem, idx):
    ev = events.get(sem)
    if not ev:
        return 0
    # cumulative value of sem right after instruction idx
    import bisect

    k = bisect.bisect_right(ev[1], idx)
    return ev[0][k - 1] if k else 0


def _append_sem_resets(nc, cleared_ranges):
    """Replace the skipped EVENT_SEMAPHORE_RANGE_CLEAR (whose raw-ISA bytes
    come from mismatched sunda tables and fail this walrus) with per-sem
    EVSEM sem-sub-imm resets appended after the final all-engine barrier.
    Final sem values are statically known (straight-line program)."""
    import concourse.mybir as mybir

    fn = nc.m.functions[0]
    blocks = list(fn.blocks)
    final = {}
    names = {}
    for b in blocks:
        for inst in b.instructions:
            si = inst.sync_info
            if si is None:
                continue
            for u in si.on_update or []:
                if u.update_mode in ("sem-inc", "sem-add-imm"):
                    final[u.id] = final.get(u.id, 0) + u.update_value
                    names[u.id] = u.ant_name
                elif u.update_mode in ("sem-dec", "sem-sub-imm"):
                    final[u.id] = final.get(u.id, 0) - u.update_value
                    names[u.id] = u.ant_name
    last = blocks[-1]
    import copy as _copy

    donor = None
    for b in blocks:
        for inst in b.instructions:
            if (
                type(inst).__name__ == "InstEventSemaphore"
                and str(inst.engine) == "EngineType.Pool"
            ):
                donor = inst
    assert donor is not None, "no Pool EVSEM to clone for sem resets"
    n = 0
    for rng in cleared_ranges:
        for sid in rng:
            v = final.get(sid, 0)
            if v <= 0:
                continue
            u = mybir.SyncUpdate(
                sync_type="semaphore", id=sid, ant_name=names.get(sid, f"sem{sid}"),
                update_mode="sem-sub-imm", update_value=v, update_reg=None,
            )
            inst = _copy.deepcopy(donor)
            inst.name = f"ant_sem_reset_{n}"
            inst.sync_info = mybir.SyncInfo(on_wait=[], on_update=[u])
            last.add_instruction(inst)
            n += 1
    return n


def _build():
    import concourse.bass as bass
    import concourse.mybir as mybir
    import concourse.tile as tile

    f32 = mybir.dt.float32
    nc = bass.Bass("TRN2", target_bir_lowering=False, debug=False, num_devices=N_CORES)
    u_aps = [
        nc.dram_tensor(f"user{k}_emb", [UL, DIM], f32, kind="ExternalInput").ap()
        for k in (1, 2, 3)
    ]
    i_aps = [
        nc.dram_tensor(f"item{k}_emb", [N_ITEMS, DIM], f32, kind="ExternalInput").ap()
        for k in (1, 2, 3)
    ]
    h_ap = nc.dram_tensor("h", [2 * DIM], f32, kind="ExternalInput").ap()
    out_ap = nc.dram_tensor("out", [UL, N_ITEMS], f32, kind="ExternalOutput").ap()

    cleared = []
    _orig_sem_clear = nc.gpsimd.sem_clear

    def _defer_sem_clear(rng):
        cleared.append(rng if isinstance(rng, range) else range(rng.num, rng.num + 1))
        return None

    nc.gpsimd.sem_clear = _defer_sem_clear
    with tile.TileContext(nc) as tc:
        _emit(tc, nc, out_ap, u_aps, i_aps, h_ap)
    nc.gpsimd.sem_clear = _orig_sem_clear
    _prune_redundant_waits(nc)
    if os.environ.get("KERNEL_SEM_RESET") == "1":
        # Leave-sems-dirty is the default: the cloned-EVSEM tail resets are
        # suspected of the EXEC_UNIT_UNRECOVERABLE crash, and a fresh NEFF
        # execution (one kernel() call) never observes the dirty end state.
        _append_sem_resets(nc, cleared)
    return nc


def _get_nc():
    if "nc" not in _CACHE:
        _CACHE["nc"] = _build()
    return _CACHE["nc"]


def _get_exec():
    """Build (once) a sharded jit callable over the 8 cores, mirroring
    bass2jax.run_bass_via_pjrt's multi-core branch but reusable for timing."""
    if "exec" in _CACHE:
        return _CACHE["exec"]
    import jax
    import concourse.mybir as mybir
    from concourse import bass2jax as B
    from jax.sharding import Mesh, PartitionSpec

    try:
        from jax.experimental.shard_map import shard_map
    except ImportError:
        from jax.shard_map import shard_map

    nc = _get_nc()
    if not _CACHE.get("same_engine_pruned"):
        _prune_redundant_waits(nc, same_engine=True)
        _CACHE["same_engine_pruned"] = True
    B.install_neuronx_cc_hook()

    partition_name = (
        nc.partition_id_tensor.name if nc.partition_id_tensor is not None else None
    )
    in_names, out_names, out_avals = [], [], []
    for alloc in nc.m.functions[0].allocations:
        if not isinstance(alloc, mybir.MemoryLocationSet):
            continue
        name = alloc.memorylocations[0].name
        if alloc.kind == "ExternalInput":
            if name != partition_name:
                in_names.append(name)
        elif alloc.kind == "ExternalOutput":
            out_names.append(name)
            out_avals.append(
                jax.core.ShapedArray(tuple(alloc.tensor_shape), mybir.dt.np(alloc.dtype))
            )
    n_params = len(in_names)
    all_names = in_names + out_names + ([partition_name] if partition_name else [])

    def _body(*args):
        operands = list(args)
        if partition_name is not None:
            operands.append(B.partition_id_tensor())
        outs = B._bass_exec_p.bind(
            *operands,
            out_avals=tuple(out_avals),
            in_names=tuple(all_names),
            out_names=tuple(out_names),
            lowering_input_output_aliases=(),
            sim_require_finite=True,
            sim_require_nnan=True,
            nc=nc,
        )
        return tuple(outs)

    devices = jax.devices()[:N_CORES]
    mesh = Mesh(np.asarray(devices), ("core",))
    n_outs = len(out_names)
    sharded = jax.jit(
        shard_map(
            _body,
            mesh=mesh,
            in_specs=(PartitionSpec("core"),) * (n_params + n_outs),
            out_specs=(PartitionSpec("core"),) * n_outs,
            check_rep=False,
        ),
        donate_argnums=tuple(range(n_params, n_params + n_outs)),
        keep_unused=True,
    )
    meta = dict(
        in_names=in_names, out_names=out_names, out_avals=out_avals, mesh=mesh
    )
    _CACHE["exec"] = (sharded, meta)
    return _CACHE["exec"]


def _concat_inputs(inputs):
    arr = {
        k: np.ascontiguousarray(np.asarray(v, dtype=np.float32))
        for k, v in inputs.items()
    }
    per_name = {}
    for name in ("user1_emb", "user2_emb", "user3_emb"):
        per_name[name] = arr[name]  # already [4096, 64]; shard_map splits axis 0
    for name in ("item1_emb", "item2_emb", "item3_emb"):
        per_name[name] = np.concatenate([arr[name]] * N_CORES, axis=0)
    per_name["h"] = np.concatenate([arr["h"]] * N_CORES, axis=0)
    return per_name


def _zeros_for(meta):
    return [
        np.zeros((N_CORES * a.shape[0], *a.shape[1:]), a.dtype)
        for a in meta["out_avals"]
    ]


def _run(inputs, timing_iters=0):
    import jax

    sharded, meta = _get_exec()
    per_name = _concat_inputs(inputs)
    args = [per_name[n] for n in meta["in_names"]]
    out_arrs = sharded(*args, *_zeros_for(meta))
    jax.block_until_ready(out_arrs)
    out = np.asarray(out_arrs[0]).reshape(-1).astype(np.float32)

    times = []
    if timing_iters > 0:
        import time
        from jax.sharding import NamedSharding, PartitionSpec

        sh = NamedSharding(meta["mesh"], PartitionSpec("core"))
        dev_args = [jax.device_put(a, sh) for a in args]
        zsets = [
            [jax.device_put(z, sh) for z in _zeros_for(meta)]
            for _ in range(timing_iters)
        ]
        jax.block_until_ready(dev_args)
        jax.block_until_ready(zsets)
        # warmup
        jax.block_until_ready(sharded(*dev_args, *[jax.device_put(z, sh) for z in _zeros_for(meta)]))
        for i in range(timing_iters):
            t0 = time.perf_counter()
            r = sharded(*dev_args, *zsets[i])
            jax.block_until_ready(r)
            times.append((time.perf_counter() - t0) * 1e9)
    return out, times


def kernel(**inputs) -> np.ndarray:
    out, _ = _run(inputs)
    return out



# revision 15
# speedup vs baseline: 86.7244x; 86.7244x over previous
"""Trainium2 Bass kernel for nn_DAMASK_GCN (attention-weighted 3-way GEMM + sigmoid).

Math: rating[n,m] = sigmoid( sum_k r_k*w_k ) with w = softmax_k(a_k),
a_k[n,m] = u_k[n].h_u + i_k[m].h_i  (separable!), r_k = u_k @ i_k^T.

Separable-softmax trick: exp(a_k) = exp(alpha_k[n]) * exp(beta_k[m]), so with
scaled embeddings  u~_k = exp(alpha_k)*u_k,  i~_k = exp(beta_k)*i_k :
    S1 = sum_k u~_k @ i~_k^T          (one K=192 GEMM)
    S0 = sum_k exp(alpha_k) exp(beta_k)^T   (one K=3 GEMM)
    rating = sigmoid(S1 / S0)
No per-cell softmax needed; everything is GEMM + cheap elementwise.

Sharding: users split across 8 cores (512 rows each); items + h replicated.
"""

import os

import numpy as np

N_USERS, N_ITEMS, DIM = 4096, 8192, 64
N_CORES = 8
UL = N_USERS // N_CORES  # 512 users per core
P = 128
NT = UL // P             # 4 user tiles per core
TS_I = N_ITEMS // P      # 64 item subtiles
GRP = 16                 # item subtiles per prep group
NG = TS_I // GRP         # 4 prep groups
QUAD = 2048              # m-columns per main-loop quad
NQ = N_ITEMS // QUAD     # 4 quads
CH = 512                 # matmul moving free dim

_CACHE = {}
USE_FUSED_DIV = False  # single custom-DVE op out = in1 * approx(1/in0) (~1.7e-3 rel)


def _register_div_op():
    """Register a fused divide custom-DVE op: out = Src1 * recip_1NR(Src0).

    Same bitwise-NOT seed + one Newton pass as RECIPROCAL_APPROX_FAST (which
    uses two passes in 8/8 stages — no room for the extra multiply), then the
    multiply by Src1. 6 stages, one DVE instruction instead of recip+mul.
    """
    if "div_op" in _CACHE:
        return _CACHE["div_op"]
    import numpy as np

    from concourse import dve_ops as DO
    from concourse.dve_spec import AluOp, Bin, C0, C1, Spec, Src0, Src1, lower
    from concourse.dve_uop import DveOpSpec

    name = "DIV_APPROX_ANT"
    _not_x = Bin(AluOp.BITWISE_NOT, Src0, Src0)
    _y0 = _not_x * C0
    _y1 = _y0 * (C1 - Src0 * _y0)

    def _ref(in0, in1, c0, c1, c2):
        not_x = (~in0.view(np.int32)).view(np.float32)
        y0 = not_x * c0
        y1 = y0 * (c1 - in0 * y0)
        return in1 * y1

    spec = Spec(body=Src1 * _y1, reference=_ref)
    row = max(DO._SUB_OPCODE_FOR_NAME.values()) + 1
    assert row < 0x20
    shas = {}
    for ver in ("v3", "v4"):
        uops = lower(spec, ver=ver)
        shas[ver] = DveOpSpec(name=name, opcode=row, uops=uops, rd1_en=True).sha(ver)
    op = DO.DveOp(name, spec, subdim=False, uops_sha=shas)
    if not any(o.name == name for o in DO.OPS):
        DO.OPS.append(op)
        DO._SUB_OPCODE_FOR_NAME[name] = row
    _CACHE["div_op"] = op
    return op


def _emit(tc, nc, out_ap, u_aps, i_aps, h_ap, reps=1):
    """Per-core program.

    Walrus's LDWEIGHTS struct encodes only ONE sync wait, and Tile does not
    exploit cross-engine transitivity -- so the structure below is arranged so
    every matmul needs at most one new semaphore wait:
      * all input DMAs are issued up front;
      * scaled embeddings live in fresh tiles whose only writers are DVE ops
        (scale multiply, then an exp-column copy as the single DVE funnel);
      * each prep group's PSUM slot-release (an ACT copy) is absorbed by a
        dummy identity transpose before the real transposes.
    """
    from contextlib import ExitStack

    import concourse.mybir as mybir
    from concourse.masks import make_identity

    f32 = mybir.dt.float32
    AF = mybir.ActivationFunctionType
    AX = mybir.AxisListType

    with ExitStack() as ctx:
        pers = ctx.enter_context(tc.tile_pool(name="pers", bufs=1))
        workA = ctx.enter_context(tc.tile_pool(name="workA", bufs=1))
        work = ctx.enter_context(tc.tile_pool(name="work", bufs=2))
        psum = ctx.enter_context(tc.tile_pool(name="psum", bufs=1, space="PSUM"))

        # ---- persistent tensors ----
        f32r = mybir.dt.float32r
        A1 = pers.tile([P, N_ITEMS], f32r)       # [i1~^T ; i2~^T] rows 0-127
        A2 = pers.tile([67, N_ITEMS], f32r)      # [i3~^T (64) ; ei1 ; ei2 ; ei3]
        B1 = pers.tile([P, UL], f32r)            # [u1~^T ; u2~^T]
        B2 = pers.tile([67, UL], f32r)           # [u3~^T (64) ; eu1 ; eu2 ; eu3]
        nat_u = pers.tile([P, NT, 192], f32)
        scl_u = pers.tile([P, NT, 195], f32r)    # scaled users + eu cols
        ident = pers.tile([P, P], f32)
        identr = pers.tile([P, P], f32r)
        hrep = pers.tile([P, 2 * DIM], f32)      # h broadcast to all partitions
        hsb = pers.tile([1, 2 * DIM], f32)
        ones1 = pers.tile([1, P], f32)

        # ---- phase 0: constants (once, outside the rep loop) ----
        from concourse.tile_rust import add_dep_helper

        nc.sync.dma_start(hsb[:], h_ap.rearrange("(a b) -> a b", a=1))
        nc.gpsimd.memset(ones1[:], 1.0)
        make_identity(nc, ident[:])

        # prime PE's Pool tick (identity/ones writers), then h-broadcast.
        f32r = mybir.dt.float32r
        ps_h = psum.tile([P, P], f32, tag="psB")
        nc.tensor.transpose(ps_h[:], ident[:], ident[:])  # dummy
        nc.tensor.matmul(ps_h[:], lhsT=ones1[:], rhs=hsb[:], start=True, stop=True)
        nc.scalar.copy(hrep[:], ps_h[:])
        nc.scalar.copy(identr[:], ident[:])

        hu_b = hrep[:, None, None, 0:DIM]
        hi_b = hrep[:, None, None, DIM : 2 * DIM]

        def prep_group(nat, scl, n_sub, gs, h_b, psA_w, psB_w, first_psum_dummy,
                       join_deps=()):
            """mult -> reduce -> exp -> scale -> ei-copy -> transposes.
            Returns (psA, psB) holding the transposed group."""
            for di in join_deps:
                jd = nc.vector.drain(fusable=False)
                add_dep_helper(jd.ins, di, sync=True, reason="dma joiner")
            nat4 = nat[:, gs, :].rearrange("p t (k d) -> p t k d", k=3)
            tmp = workA.tile([P, n_sub, 3, DIM], f32, tag="tmp")
            nc.vector.tensor_mul(tmp[:], nat4, h_b.to_broadcast([P, n_sub, 3, DIM]))
            bt = workA.tile([P, n_sub, 3], f32, tag="bt")
            nc.vector.reduce_sum(bt[:], tmp[:], axis=AX.X)
            bt2 = workA.tile([P, n_sub, 3], f32, tag="bt2")
            nc.scalar.activation(bt2[:], bt[:], AF.Exp)
            scl4 = scl[:, gs, 0:192].rearrange("p t (k d) -> p t k d", k=3)
            nc.vector.tensor_mul(
                scl4, nat4, bt2[:][:, :, :, None].to_broadcast([P, n_sub, 3, DIM])
            )
            # single DVE funnel: after this, scl is fully written by DVE only
            nc.vector.tensor_copy(scl[:, gs, 192:195], bt2[:])

            psA = psum.tile([P, psA_w], f32r, tag="psA")
            psB = psum.tile([P, psB_w], f32r, tag="psB")
            if first_psum_dummy:
                # absorb the ACT slot-release wait on PE before real work
                nc.tensor.transpose(psB[:, 0:P], identr[:], identr[:])
            for j in range(n_sub):
                t = gs.start + j
                nc.tensor.transpose(
                    psB[0:67, j * P : (j + 1) * P], scl[:, t, 128:195],
                    identr[:],
                )
                nc.tensor.transpose(
                    psA[:, j * P : (j + 1) * P], scl[:, t, 0:128],
                    identr[:],
                )
            return psA, psB

        all_tp = []          # item-group last-transpose handles, global across reps
        _sig_ins = []        # sigmoid handles, global across reps
        _dma_ins = []        # out-DMA handles, global across reps

        for rep in range(reps):
            # ---- per-rep input DMAs (users) ----
            _udmas = []
            for k in range(3):
                d = nc.sync.dma_start(
                    nat_u[:, :, k * DIM : (k + 1) * DIM],
                    u_aps[k].rearrange("(t p) d -> p t d", p=P),
                )
                _udmas.append(d.ins)

            # ---- user prep ----
            psU, psU2 = prep_group(
                nat_u, scl_u, NT, slice(0, NT), hu_b, CH, CH, first_psum_dummy=True,
                join_deps=_udmas,
            )
            nc.scalar.copy(B1[:], psU[:])
            nc.scalar.copy(B2[:], psU2[0:67, :])

            # ---- item prep (NG groups) ----
            for g in range(NG):
                gi = rep * NG + g
                scl_i = work.tile([P, GRP, 195], f32r, tag="scl_i")
                nat_g = work.tile([P, GRP, 192], f32, tag="nat_g")
                gs_nat = slice(g * GRP, (g + 1) * GRP)
                _idmas = []
                for k in range(3):
                    d = nc.sync.dma_start(
                        nat_g[:, :, k * DIM : (k + 1) * DIM],
                        i_aps[k].rearrange("(t p) d -> p t d", p=P)[:, gs_nat, :],
                    )
                    _idmas.append(d.ins)
                if gi >= 2:
                    _idmas.append(all_tp[gi - 2])
                for di in _idmas:
                    jd = nc.vector.drain(fusable=False)
                    add_dep_helper(jd.ins, di, sync=True, reason="group joiner")

                nat4 = nat_g[:, :, :].rearrange("p t (k d) -> p t k d", k=3)
                tmp = workA.tile([P, GRP, 3, DIM], f32, tag="tmp")
                nc.vector.tensor_mul(tmp[:], nat4, hi_b.to_broadcast([P, GRP, 3, DIM]))
                bt = workA.tile([P, GRP, 3], f32, tag="bt")
                nc.vector.reduce_sum(bt[:], tmp[:], axis=AX.X)
                bt2 = workA.tile([P, GRP, 3], f32, tag="bt2")
                nc.scalar.activation(bt2[:], bt[:], AF.Exp)
                scl4 = scl_i[:, :, 0:192].rearrange("p t (k d) -> p t k d", k=3)
                nc.vector.tensor_mul(
                    scl4, nat4, bt2[:][:, :, :, None].to_broadcast([P, GRP, 3, DIM])
                )
                nc.vector.tensor_copy(scl_i[:, :, 192:195], bt2[:])

                psA = psum.tile([P, QUAD], f32r, tag="psA")
                psB = psum.tile([P, QUAD], f32r, tag="psB")
                nc.tensor.transpose(psB[:, 0:P], identr[:], identr[:])  # slot absorber
                for j in range(GRP):
                    nc.tensor.transpose(
                        psB[0:67, j * P : (j + 1) * P], scl_i[:, j, 128:195],
                        identr[:],
                    )
                    tp = nc.tensor.transpose(
                        psA[:, j * P : (j + 1) * P], scl_i[:, j, 0:128],
                        identr[:],
                    )
                all_tp.append(tp.ins)
                nc.scalar.copy(A1[:, g * QUAD : (g + 1) * QUAD], psA[:])
                nc.scalar.copy(A2[:, g * QUAD : (g + 1) * QUAD], psB[0:67, :])

            # ---- main loop ----
            for t in range(NT):
                tsl = slice(t * P, (t + 1) * P)
                b1 = B1[:, tsl]
                b2e = B2[0:64, tsl]
                b2s = B2[64:67, tsl]
                for q in range(NQ):
                    ps0 = psum.tile([P, QUAD], f32, tag="psB")
                    for c in range(QUAD // CH):
                        off = q * QUAD + c * CH
                        nc.tensor.matmul(
                            ps0[:, c * CH : (c + 1) * CH],
                            lhsT=b2s,
                            rhs=A2[64:67, off : off + CH],
                            start=True,
                            stop=True,
                        )
                    r0 = workA.tile([P, QUAD], f32, tag="r0")
                    nc.vector.reciprocal(r0[:], ps0[:])
                    ps1 = psum.tile([P, QUAD], f32, tag="psA")
                    for c in range(QUAD // CH):
                        off = q * QUAD + c * CH
                        cs = slice(c * CH, (c + 1) * CH)
                        nc.tensor.matmul(
                            ps1[:, cs], lhsT=b1, rhs=A1[:, off : off + CH],
                            start=True, stop=False,
                        )
                        nc.tensor.matmul(
                            ps1[:, cs], lhsT=b2e, rhs=A2[0:64, off : off + CH],
                            start=False, stop=True,
                        )
                    qi = rep * NT * NQ + t * NQ + q
                    if qi >= 2:
                        jd = nc.vector.drain(fusable=False)
                        add_dep_helper(jd.ins, _sig_ins[qi - 2], sync=True, reason="tq slot")
                        ja = nc.scalar.drain(fusable=False)
                        add_dep_helper(ja.ins, _dma_ins[qi - 2], sync=True, reason="oq slot")
                    tq = work.tile([P, QUAD], f32, tag="tq")
                    nc.vector.tensor_mul(tq[:], ps1[:], r0[:])
                    oq = work.tile([P, QUAD], f32, tag="oq")
                    s = nc.scalar.activation(oq[:], tq[:], AF.Sigmoid)
                    _sig_ins.append(s.ins)
                    d = nc.sync.dma_start(out_ap[tsl, q * QUAD : (q + 1) * QUAD], oq[:])
                    _dma_ins.append(d.ins)

        # tail: let SP observe every DMA queue's final tick so the framework
        # drain's multi-queue waits prune to <=1 (walrus wait-slot limit)
        for di in _dma_ins[-8:]:
            jd = nc.sync.drain(fusable=False)
            add_dep_helper(jd.ins, di, sync=True, reason="tail dma observe")


def _prune_redundant_waits(nc, same_engine=False):
    """Transitive wait elimination.

    Walrus can encode only ONE sync wait on a Matmult (all waits land on the
    LDWEIGHTS struct), and Tile's per-instruction wait assignment is not
    transitively minimal across engines. Nearly every extra wait here is
    implied: e.g. a matmul waiting {ACT>=c, PE>=t} where the ACT copy at tick c
    itself waited PE>=t. This pass simulates the scheduled stream with vector
    clocks and deletes waits that are (a) transitively implied by kept waits /
    the engine stream, or (b) same-engine completion waits on in-order
    pipelined engines (PE/DVE/ACT), whose writes are ordered by hardware.
    """
    import bisect

    import concourse.mybir as mybir

    fn = nc.m.functions[0]
    insts = [i for blk in fn.blocks for i in blk.instructions]
    idx_of = {inst.name: n for n, inst in enumerate(insts)}

    # sems ever decremented/reset are untouchable (drain-tail butterfly).
    # Our own appended tail resets (ant_sem_reset_*) are after the final
    # barrier and don't affect reasoning — skip them here.
    insts = [i for i in insts if not i.name.startswith("ant_sem_reset")]
    blacklist = set()
    for inst in insts:
        si = inst.sync_info
        if si is None:
            continue
        for u in si.on_update or []:
            if u.update_mode not in ("sem-inc", "sem-add-imm"):
                blacklist.add(u.ant_name)

    # sem event lists: name -> ([cum_value...], [inst_idx...]) in stream order
    events = {}
    cum = {}
    for n, inst in enumerate(insts):
        si = inst.sync_info
        if si is None:
            continue
        for u in si.on_update or []:
            name = u.ant_name
            if name in blacklist:
                continue
            cum[name] = cum.get(name, 0) + u.update_value
            events.setdefault(name, ([], []))
            events[name][0].append(cum[name])
            events[name][1].append(n)

    def guarantor(sem, v):
        ev = events.get(sem)
        if ev is None:
            return None
        k = bisect.bisect_left(ev[0], v)
        if k >= len(ev[0]):
            return None
        return ev[1][k]

    SAME_ENGINE_OK = {"PE": "EngineType.PE", "DVE": "EngineType.DVE",
                      "Activation": "EngineType.Activation",
                      "SP": "EngineType.SP"}

    def sem_engine(sem):
        return sem.rsplit("_", 1)[0]

    def stream_of(inst):
        # DMAs on the same software lane (DMAHWn sem) retire in FIFO order on
        # the single qSPDynamicHW ring: same lane -> same tile-slot stream ->
        # identical descriptor split, so per-SDMA-lane ordering covers WAW.
        if type(inst).__name__ == "InstDMACopy" and inst.sync_info is not None:
            for u in inst.sync_info.on_update or []:
                if u.ant_name.startswith("DMAHW"):
                    return "DMAQ:" + sem_engine(u.ant_name)
        return str(inst.engine)

    stream_clock = {}      # engine -> dict(sem -> guaranteed value at next start)
    completion = [None] * len(insts)  # per-inst completion clock

    def join(dst, src):
        for k, v in src.items():
            if dst.get(k, 0) < v:
                dst[k] = v

    n_del = 0
    for n, inst in enumerate(insts):
        eng = stream_of(inst)
        start = dict(stream_clock.get(eng, {}))
        si = inst.sync_info
        kept = []
        if si is not None and si.on_wait:
            waits = list(si.on_wait)
            # resolve guarantors; process latest-guarantor-first so broad
            # clocks are merged before testing narrower waits
            gids = [(w, guarantor(w.ant_name, w.wait_value)) for w in waits]
            gids.sort(key=lambda t: -1 if t[1] is None else -t[1])
            for w, g in gids:
                if w.ant_name in blacklist:
                    kept.append(w)  # non-monotonic sem: never reason about it
                    continue
                if start.get(w.ant_name, 0) >= w.wait_value:
                    n_del += 1
                    continue
                se = sem_engine(w.ant_name)
                if (
                    same_engine
                    and (
                        SAME_ENGINE_OK.get(se) == eng
                        or (se.startswith("DMAHW") and eng == "DMAQ:" + se)
                    )
                    and g is not None
                    and g < n
                ):
                    # in-order pipelined engine: own-queue completion order
                    # makes this wait redundant on hardware (CoreSim's race
                    # model still wants it, so this runs on the HW copy only)
                    if completion[g] is not None:
                        join(start, completion[g])
                    start[w.ant_name] = max(start.get(w.ant_name, 0), w.wait_value)
                    n_del += 1
                    continue
                kept.append(w)
                if g is not None and g < n and completion[g] is not None:
                    join(start, completion[g])
                start[w.ant_name] = max(start.get(w.ant_name, 0), w.wait_value)
            if len(kept) != len(waits):
                inst.sync_info = mybir.SyncInfo(on_wait=kept, on_update=si.on_update)
        comp = dict(start)
        if si is not None:
            for u in si.on_update or []:
                comp[u.ant_name] = max(comp.get(u.ant_name, 0), cum_at(events, u.ant_name, n))
        completion[n] = comp
        stream_clock[eng] = start
    return n_del


def _split_excess_waits(nc):
    """Insert same-engine InstDrain clones before any instruction left with
    >1 sync waits after pruning, each drain carrying one extra wait. Engine
    stream order then guarantees the drains retire first, so the victim
    needs only its last wait (walrus encodes at most one sync wait per
    instruction). DMA queue instructions can't be fixed this way (they
    execute on DGE queues, not an engine stream) — returns their count."""
    import copy as _copy

    import concourse.mybir as mybir

    fn = nc.m.functions[0]
    donor = None
    for b in fn.blocks:
        for inst in b.instructions:
            if type(inst).__name__ == "InstDrain":
                donor = inst
                break
        if donor is not None:
            break
    assert donor is not None, "no InstDrain donor to clone"
    unfixed = 0
    n = 0
    for b in fn.blocks:
        out = []
        changed = False
        for inst in b.instructions:
            si = inst.sync_info
            if si is not None and si.on_wait and len(si.on_wait) > 1:
                if type(inst).__name__ == "InstDMACopy":
                    unfixed += 1
                    out.append(inst)
                    continue
                changed = True
                waits = list(si.on_wait)
                for w in waits[:-1]:
                    dr = _copy.deepcopy(donor)
                    dr.name = f"ant_split_wait_{n}"
                    n += 1
                    dr.engine = inst.engine
                    dr.sync_info = mybir.SyncInfo(on_wait=[w], on_update=[])
                    out.append(dr)
                inst.sync_info = mybir.SyncInfo(
                    on_wait=[waits[-1]], on_update=si.on_update
                )
            out.append(inst)
        if changed:
            b.instructions = out
    return unfixed


def cum_at(events, sem, idx):
    ev = events.get(sem)
    if not ev:
        return 0
    # cumulative value of sem right after instruction idx
    import bisect

    k = bisect.bisect_right(ev[1], idx)
    return ev[0][k - 1] if k else 0


def _append_sem_resets(nc, cleared_ranges):
    """Replace the skipped EVENT_SEMAPHORE_RANGE_CLEAR (whose raw-ISA bytes
    come from mismatched sunda tables and fail this walrus) with per-sem
    EVSEM sem-sub-imm resets appended after the final all-engine barrier.
    Final sem values are statically known (straight-line program)."""
    import concourse.mybir as mybir

    fn = nc.m.functions[0]
    blocks = list(fn.blocks)
    final = {}
    names = {}
    for b in blocks:
        for inst in b.instructions:
            si = inst.sync_info
            if si is None:
                continue
            for u in si.on_update or []:
                if u.update_mode in ("sem-inc", "sem-add-imm"):
                    final[u.id] = final.get(u.id, 0) + u.update_value
                    names[u.id] = u.ant_name
                elif u.update_mode in ("sem-dec", "sem-sub-imm"):
                    final[u.id] = final.get(u.id, 0) - u.update_value
                    names[u.id] = u.ant_name
    last = blocks[-1]
    import copy as _copy

    donor = None
    for b in blocks:
        for inst in b.instructions:
            if (
                type(inst).__name__ == "InstEventSemaphore"
                and str(inst.engine) == "EngineType.Pool"
            ):
                donor = inst
    assert donor is not None, "no Pool EVSEM to clone for sem resets"
    n = 0
    for rng in cleared_ranges:
        for sid in rng:
            v = final.get(sid, 0)
            if v <= 0:
                continue
            u = mybir.SyncUpdate(
                sync_type="semaphore", id=sid, ant_name=names.get(sid, f"sem{sid}"),
                update_mode="sem-sub-imm", update_value=v, update_reg=None,
            )
            inst = _copy.deepcopy(donor)
            inst.name = f"ant_sem_reset_{n}"
            inst.sync_info = mybir.SyncInfo(on_wait=[], on_update=[u])
            last.add_instruction(inst)
            n += 1
    return n


def _build(reps=1):
    import concourse.bass as bass
    import concourse.mybir as mybir
    import concourse.tile as tile

    f32 = mybir.dt.float32
    nc = bass.Bass("TRN2", target_bir_lowering=False, debug=False, num_devices=N_CORES)
    u_aps = [
        nc.dram_tensor(f"user{k}_emb", [UL, DIM], f32, kind="ExternalInput").ap()
        for k in (1, 2, 3)
    ]
    i_aps = [
        nc.dram_tensor(f"item{k}_emb", [N_ITEMS, DIM], f32, kind="ExternalInput").ap()
        for k in (1, 2, 3)
    ]
    h_ap = nc.dram_tensor("h", [2 * DIM], f32, kind="ExternalInput").ap()
    out_ap = nc.dram_tensor("out", [UL, N_ITEMS], f32, kind="ExternalOutput").ap()

    cleared = []
    _orig_sem_clear = nc.gpsimd.sem_clear

    def _defer_sem_clear(rng):
        cleared.append(rng if isinstance(rng, range) else range(rng.num, rng.num + 1))
        return None

    nc.gpsimd.sem_clear = _defer_sem_clear
    with tile.TileContext(nc) as tc:
        _emit(tc, nc, out_ap, u_aps, i_aps, h_ap, reps=reps)
    nc.gpsimd.sem_clear = _orig_sem_clear
    _prune_redundant_waits(nc)
    if os.environ.get("KERNEL_SEM_RESET") == "1":
        # Leave-sems-dirty is the default: the cloned-EVSEM tail resets are
        # suspected of the EXEC_UNIT_UNRECOVERABLE crash, and a fresh NEFF
        # execution (one kernel() call) never observes the dirty end state.
        _append_sem_resets(nc, cleared)
    return nc


def _get_nc(reps=1):
    key = ("nc", reps)
    if key not in _CACHE:
        _CACHE[key] = _build(reps)
    return _CACHE[key]


def _get_exec(reps=1, donate=True):
    """Build (once per reps) a sharded jit callable over the 8 cores, mirroring
    bass2jax.run_bass_via_pjrt's multi-core branch but reusable for timing.
    donate=False lets one set of output buffers be reused across timing calls."""
    ekey = ("exec", reps, donate)
    if ekey in _CACHE:
        return _CACHE[ekey]
    import jax
    import concourse.mybir as mybir
    from concourse import bass2jax as B
    from jax.sharding import Mesh, PartitionSpec

    try:
        from jax.experimental.shard_map import shard_map
    except ImportError:
        from jax.shard_map import shard_map

    nc = _get_nc(reps)
    pkey = ("same_engine_pruned", reps)
    if not _CACHE.get(pkey):
        _prune_redundant_waits(nc, same_engine=True)
        n_unfixed = _split_excess_waits(nc)
        assert n_unfixed == 0, f"{n_unfixed} multi-wait DMA instructions remain"
        _CACHE[pkey] = True
    B.install_neuronx_cc_hook()

    partition_name = (
        nc.partition_id_tensor.name if nc.partition_id_tensor is not None else None
    )
    in_names, out_names, out_avals = [], [], []
    for alloc in nc.m.functions[0].allocations:
        if not isinstance(alloc, mybir.MemoryLocationSet):
            continue
        name = alloc.memorylocations[0].name
        if alloc.kind == "ExternalInput":
            if name != partition_name:
                in_names.append(name)
        elif alloc.kind == "ExternalOutput":
            out_names.append(name)
            out_avals.append(
                jax.core.ShapedArray(tuple(alloc.tensor_shape), mybir.dt.np(alloc.dtype))
            )
    n_params = len(in_names)
    all_names = in_names + out_names + ([partition_name] if partition_name else [])

    def _body(*args):
        operands = list(args)
        if partition_name is not None:
            operands.append(B.partition_id_tensor())
        outs = B._bass_exec_p.bind(
            *operands,
            out_avals=tuple(out_avals),
            in_names=tuple(all_names),
            out_names=tuple(out_names),
            lowering_input_output_aliases=(),
            sim_require_finite=True,
            sim_require_nnan=True,
            nc=nc,
        )
        return tuple(outs)

    devices = jax.devices()[:N_CORES]
    mesh = Mesh(np.asarray(devices), ("core",))
    n_outs = len(out_names)
    sharded = jax.jit(
        shard_map(
            _body,
            mesh=mesh,
            in_specs=(PartitionSpec("core"),) * (n_params + n_outs),
            out_specs=(PartitionSpec("core"),) * n_outs,
            check_rep=False,
        ),
        donate_argnums=(
            tuple(range(n_params, n_params + n_outs)) if donate else ()
        ),
        keep_unused=True,
    )
    meta = dict(
        in_names=in_names, out_names=out_names, out_avals=out_avals, mesh=mesh
    )
    _CACHE[ekey] = (sharded, meta)
    return _CACHE[ekey]


def _concat_inputs(inputs):
    arr = {
        k: np.ascontiguousarray(np.asarray(v, dtype=np.float32))
        for k, v in inputs.items()
    }
    per_name = {}
    for name in ("user1_emb", "user2_emb", "user3_emb"):
        per_name[name] = arr[name]  # already [4096, 64]; shard_map splits axis 0
    for name in ("item1_emb", "item2_emb", "item3_emb"):
        per_name[name] = np.concatenate([arr[name]] * N_CORES, axis=0)
    per_name["h"] = np.concatenate([arr["h"]] * N_CORES, axis=0)
    return per_name


def _zeros_for(meta):
    return [
        np.zeros((N_CORES * a.shape[0], *a.shape[1:]), a.dtype)
        for a in meta["out_avals"]
    ]


def _run(inputs, timing_iters=0):
    import jax

    sharded, meta = _get_exec()
    per_name = _concat_inputs(inputs)
    args = [per_name[n] for n in meta["in_names"]]
    out_arrs = sharded(*args, *_zeros_for(meta))
    jax.block_until_ready(out_arrs)
    out = np.asarray(out_arrs[0]).reshape(-1).astype(np.float32)

    times = []
    if timing_iters > 0:
        import time
        from jax.sharding import NamedSharding, PartitionSpec

        sh = NamedSharding(meta["mesh"], PartitionSpec("core"))
        dev_args = [jax.device_put(a, sh) for a in args]
        zsets = [
            [jax.device_put(z, sh) for z in _zeros_for(meta)]
            for _ in range(timing_iters)
        ]
        jax.block_until_ready(dev_args)
        jax.block_until_ready(zsets)
        # warmup
        jax.block_until_ready(sharded(*dev_args, *[jax.device_put(z, sh) for z in _zeros_for(meta)]))
        for i in range(timing_iters):
            t0 = time.perf_counter()
            r = sharded(*dev_args, *zsets[i])
            jax.block_until_ready(r)
            times.append((time.perf_counter() - t0) * 1e9)
    return out, times


def measure_hw_exec_ns(inputs, reps_lo=1, reps_hi=65, trials=16):
    """Steady-state per-iteration HW execution time of the kernel.

    One NEFF executes the full kernel body (input DMAs, prep, GEMMs,
    softmax/sigmoid, output DMAs) `reps` times back-to-back on device. The
    slope between the reps_lo and reps_hi walls cancels the client->terminal
    tunnel round-trip (~70 ms here) and all per-launch overheads, leaving
    pure device execution time per kernel iteration. Trials for the two
    configs are interleaved so both sample the same tunnel-latency window
    (the WAN floor drifts by several ms over minutes).
    """
    import time

    import jax
    from jax.sharding import NamedSharding, PartitionSpec

    per_name = _concat_inputs(inputs)
    execs = {}
    for reps in (reps_lo, reps_hi):
        sharded, meta = _get_exec(reps, donate=False)
        sh = NamedSharding(meta["mesh"], PartitionSpec("core"))
        args = [jax.device_put(per_name[n], sh) for n in meta["in_names"]]
        zs = [jax.device_put(z, sh) for z in _zeros_for(meta)]
        jax.block_until_ready(args)
        jax.block_until_ready(zs)
        jax.block_until_ready(sharded(*args, *zs))  # compile + NEFF load
        execs[reps] = (sharded, args, zs)
    mins = {reps_lo: None, reps_hi: None}
    for _ in range(trials):
        for reps in (reps_lo, reps_hi):
            sharded, args, zs = execs[reps]
            t0 = time.perf_counter()
            r = sharded(*args, *zs)
            jax.block_until_ready(r)
            dt = (time.perf_counter() - t0) * 1e9
            if mins[reps] is None or dt < mins[reps]:
                mins[reps] = dt
    per_iter = (mins[reps_hi] - mins[reps_lo]) / (reps_hi - reps_lo)
    return per_iter, mins


def kernel(**inputs) -> np.ndarray:
    out, _ = _run(inputs)
    return out

